# revision 14
# baseline (speedup 1.0000x reference)
"""TRN2 Bass kernel for nn_AttentionModule (dense transformer attention block).

Reference computation (per sample b, x flattened to [256, 4096]):
    proj = conv_w @ x + conv_b                 [32, 4096]
    q    = (q_w @ proj + q_b).T                [4096, 32]
    k    = k_w @ proj + k_b                    [32, 4096]
    v    = v_w @ proj + v_b                    [256, 4096]
    attn = softmax(q @ k, axis=-1)             [4096(n), 4096(m)]
    out  = gamma * (v @ attn.T) + x            [256, 4096]

Sharding: 8 cores = 4 samples x 2 query-halves (2048 queries each). Each core
redundantly computes proj/k/v for its sample (cheap) and its half of the
queries. No cross-core communication. SPMD: odd cores receive x with the
spatial axis rolled by -2048 so "their" queries sit at columns 0:2048;
attention is permutation-invariant over keys so k/v column order is free.

On-core layout: scores are computed transposed, [m_keys(part), n_queries
(free)], so the exp'd scores chunks are directly usable as matmul weights
(lhsT) for the attn@V contraction over m, and the softmax denominator falls
out of the same matmul via an appended ones-column in the V^T projection
(column 256 of the [33,257] rhs; proj carries a ones-row 32 that also folds
in the v bias). No max-subtraction: exp'd scores are stored in bf16 (no
overflow below e^88); numerator and denominator share the same bf16 rounding
so softmax normalization cancels most of it. The residual is applied in
[n, c] layout against a host-transposed x, and the host transposes the
[2048, 256] per-core output back — no on-chip transposes at all.

gamma is folded into v_w/v_b host-side. fp16 feeds the q/k score path.
"""

import numpy as np
from contextlib import ExitStack

import concourse.bass as bass
import concourse.bacc as bacc
import concourse.tile as tile
from concourse import mybir
from concourse.bass_utils import run_bass_kernel_spmd

F32 = mybir.dt.float32
F16 = mybir.dt.float16
BF16 = mybir.dt.bfloat16

B, C, H, W = 4, 256, 64, 64
HW = H * W          # 4096 keys (m)
NQ = HW // 2        # 2048 queries per core (n)
C8 = 32             # qk head dim (e) / proj channels (d)
NSUP = 512          # queries per attention super-block
NBLK = 128          # queries per attnout block
MCH = 128           # keys per m-chunk (one lhsT tile)
N_MCH = HW // MCH   # 32 m-chunks
VN = C + 1          # 257: v channels + ones column (softmax denominator)

_CACHED = {}


def build_nc():
    nc = bacc.Bacc("TRN2", target_bir_lowering=False, debug=False)
    d_x16 = nc.dram_tensor("x16", [C, HW], F16, kind="ExternalInput").ap()
    d_xT = nc.dram_tensor("xT", [NQ, C], F32, kind="ExternalInput").ap()
    d_cwT = nc.dram_tensor("cwT", [2, 128, C8], F16, kind="ExternalInput").ap()
    d_cb = nc.dram_tensor("cb", [C8, 1], F32, kind="ExternalInput").ap()
    d_kwT = nc.dram_tensor("kwT", [C8, C8], F16, kind="ExternalInput").ap()
    d_kb4 = nc.dram_tensor("kb4", [128, 1], F32, kind="ExternalInput").ap()
    d_qwT = nc.dram_tensor("qwT", [C8, C8], F16, kind="ExternalInput").ap()
    d_qb4 = nc.dram_tensor("qb4", [128, 1], F32, kind="ExternalInput").ap()
    d_vwb = nc.dram_tensor("vwb", [C8 + 1, VN], F16, kind="ExternalInput").ap()
    d_outT = nc.dram_tensor("outT", [NQ, C], F32, kind="ExternalOutput").ap()

    with tile.TileContext(nc) as tc, ExitStack() as ctx:
        const_pool = ctx.enter_context(tc.tile_pool(name="const", bufs=1))
        big_pool = ctx.enter_context(tc.tile_pool(name="big", bufs=1))

        # ---- constants / inputs ----
        cwT = const_pool.tile([128, 2, C8], F16)
        kwT = const_pool.tile([C8, C8], F16)
        qwT = const_pool.tile([C8, C8], F16)
        vwb = const_pool.tile([C8 + 1, VN], F16)
        cb = const_pool.tile([C8, 1], F32)
        kb4 = const_pool.tile([128, 1], F32)
        qb4 = const_pool.tile([128, 1], F32)
        for a in range(2):
            nc.sync.dma_start(cwT[:, a, :], d_cwT[a])
        nc.sync.dma_start(kwT[:], d_kwT)
        nc.sync.dma_start(qwT[:], d_qwT)
        nc.sync.dma_start(vwb[:], d_vwb)
        nc.sync.dma_start(cb[:], d_cb)
        nc.sync.dma_start(kb4[:], d_kb4)
        nc.sync.dma_start(qb4[:], d_qb4)

        # x16: two c-halves [128, HW] fp16 (matmul operand)
        x16 = [big_pool.tile([128, HW], F16, tag=f"x16_{i}", name=f"x16_{i}") for i in range(2)]
        d_x16v = d_x16.rearrange("(a p) m -> a p m", p=128)
        for i in range(2):
            for j in range(4):
                sl = bass.ts(j, HW // 4)
                nc.sync.dma_start(x16[i][:, sl], d_x16v[i][:, sl])

        # xT: residual input, [128, nb, 256]: query block nb on partitions
        xT = big_pool.tile([128, NQ // NBLK, C], F32)
        d_xTv = d_xT.rearrange("(nb p) c -> p nb c", p=128)
        for j in range(4):
            nbs = NQ // NBLK // 4
            nc.sync.dma_start(xT[:, j * nbs : (j + 1) * nbs, :],
                              d_xTv[:, j * nbs : (j + 1) * nbs, :])

        proj = big_pool.tile([C8 + 1, HW], F16)   # row 32 = ones
        nc.gpsimd.memset(proj[C8 : C8 + 1, :], 1.0)
        k4 = big_pool.tile([128, HW], F16)        # k replicated on 4 row-groups
        qT4 = big_pool.tile([128, NQ], F16)       # query half, replicated x4
        vt = big_pool.tile([128, N_MCH * VN], BF16)  # vT' chunks [m=128, 257]

        # ---- projections ----
        with tc.tile_pool(name="psA", bufs=2, space="PSUM") as psA:
            # proj = conv_w @ x + conv_b  (K = 256 over 2 chunks)
            for hblk in range(2):
                pp = psA.tile([C8, HW // 2], F32, tag="pp")
                for j in range(4):
                    sl = bass.ts(j, 512)
                    gsl = bass.ds(hblk * (HW // 2) + j * 512, 512)
                    nc.tensor.matmul(pp[:, sl], cwT[:, 0, :], x16[0][:, gsl],
                                     start=True, stop=False)
                    nc.tensor.matmul(pp[:, sl], cwT[:, 1, :], x16[1][:, gsl],
                                     start=False, stop=True)
                nc.vector.tensor_scalar_add(
                    proj[0:C8, bass.ts(hblk, HW // 2)], pp[:], cb[:])

        with tc.tile_pool(name="psB", bufs=2, space="PSUM") as psB:
            # k4 = k_w @ proj + k_b on all 4 col-groups (x4 replication)
            for hblk in range(2):
                pk = psB.tile([128, HW // 2], F32, tag="pk")
                for j in range(4):
                    sl = bass.ts(j, 512)
                    gsl = bass.ds(hblk * (HW // 2) + j * 512, 512)
                    for g in range(4):
                        nc.tensor.matmul(pk[bass.ts(g, 32), sl], kwT[:],
                                         proj[0:C8, gsl],
                                         tile_position=(0, 32 * g))
                nc.vector.tensor_scalar_add(
                    k4[:, bass.ts(hblk, HW // 2)], pk[:], kb4[:])

        with tc.tile_pool(name="psC1", bufs=1, space="PSUM") as psC1:
            # qT4 = q_w @ proj[:, 0:2048] + q_b on all 4 col-groups
            pq = psC1.tile([128, NQ], F32, tag="pq")
            for j in range(4):
                sl = bass.ts(j, 512)
                for g in range(4):
                    nc.tensor.matmul(pq[bass.ts(g, 32), sl], qwT[:],
                                     proj[0:C8, sl],
                                     tile_position=(0, 32 * g))
            nc.vector.tensor_scalar_add(qT4[:], pq[:], qb4[:])

        with tc.tile_pool(name="psC2", bufs=2, space="PSUM") as psC2:
            # vT' chunks: [m 128, 257] = proj'[:, chunk].T @ vwb
            for mi in range(N_MCH):
                pv = psC2.tile([128, VN], F32, tag="pv")
                nc.tensor.matmul(pv[:], proj[:, bass.ts(mi, MCH)], vwb[:])
                nc.vector.tensor_copy(vt[:, bass.ts(mi, VN)], pv[:])

        # ---- attention ----
        att_pool = ctx.enter_context(tc.tile_pool(name="att", bufs=2))
        out_pool = ctx.enter_context(tc.tile_pool(name="outp", bufs=3))
        ps_s = ctx.enter_context(tc.tile_pool(name="ps_s", bufs=1, space="PSUM"))
        ps_o = ctx.enter_context(tc.tile_pool(name="ps_o", bufs=4, space="PSUM"))

        for ns in range(NQ // NSUP):          # 4 n-super-blocks of 512 queries
            nsl = bass.ts(ns, NSUP)
            e_sb = att_pool.tile([128, N_MCH * NSUP], BF16, tag="e_sb")
            for grp in range(N_MCH // 4):     # 8 groups of 4 m-chunks
                ps = ps_s.tile([128, 4 * NSUP], F32, tag="ps")  # 4 banks
                for i in range(4):
                    mi = 4 * grp + i
                    nc.tensor.matmul(
                        ps[:, bass.ts(i, NSUP)],
                        k4[bass.ts(i, 32), bass.ts(mi, MCH)],
                        qT4[bass.ts(i, 32), nsl],
                        tile_position=(32 * i, 0),
                    )
                # exp over the whole 4-bank group in one ACT op
                nc.scalar.activation(
                    e_sb[:, bass.ds(4 * grp * NSUP, 4 * NSUP)], ps[:],
                    mybir.ActivationFunctionType.Exp)

            for nb in range(NSUP // NBLK):    # 4 blocks of 128 queries
                po = ps_o.tile([128, VN], F32, tag="po")
                for mi in range(N_MCH):
                    nc.tensor.matmul(
                        po[:],
                        e_sb[:, bass.ds(mi * NSUP + nb * NBLK, NBLK)],
                        vt[:, bass.ts(mi, VN)],
                        start=(mi == 0), stop=(mi == N_MCH - 1),
                    )
                nbg = ns * (NSUP // NBLK) + nb   # global block index
                rcol = out_pool.tile([128, 1], F32, tag="rcol")
                nc.vector.reciprocal(rcol[:], po[:, C : C + 1])
                anorm = out_pool.tile([128, C], F32, tag="anorm")
                nc.vector.tensor_scalar_mul(anorm[:], po[:, 0:C], rcol[:])
                osb = out_pool.tile([128, C], F32, tag="osb")
                nc.vector.tensor_add(osb[:], anorm[:], xT[:, nbg, :])
                nc.sync.dma_start(
                    d_outT.rearrange("(nb p) c -> p nb c", p=128)[:, nbg, :],
                    osb[:])

    nc.compile()
    return nc


def _prep_in_maps(x, conv_w, conv_b, q_w, q_b, k_w, k_b, v_w, v_b, gamma):
    g = np.float32(gamma[0])
    cwT = np.ascontiguousarray(conv_w.T.reshape(2, 128, C8)).astype(np.float16)
    kwT = np.ascontiguousarray(k_w.T).astype(np.float16)
    qwT = np.ascontiguousarray(q_w.T).astype(np.float16)
    vwb = np.zeros((C8 + 1, VN), np.float16)
    vwb[0:C8, 0:C] = (g * v_w).T.astype(np.float16)
    vwb[C8, 0:C] = (g * v_b).astype(np.float16)
    vwb[C8, C] = 1.0
    cb = conv_b.reshape(C8, 1).astype(np.float32)
    kb4 = np.tile(k_b, 4).reshape(128, 1).astype(np.float32)
    qb4 = np.tile(q_b, 4).reshape(128, 1).astype(np.float32)

    in_maps = []
    for core in range(8):
        b, hf = core // 2, core % 2
        xf = np.asarray(x[b], np.float32).reshape(C, HW)
        if hf:
            # rotate spatial columns: this core's query half -> cols 0:2048
            xf = np.roll(xf, -NQ, axis=1)
        in_maps.append({
            "x16": np.ascontiguousarray(xf).astype(np.float16),
            "xT": np.ascontiguousarray(xf[:, 0:NQ].T),
            "cwT": cwT, "cb": cb, "kwT": kwT, "kb4": kb4,
            "qwT": qwT, "qb4": qb4, "vwb": vwb,
        })
    return in_maps


def kernel(x, conv_w, conv_b, q_w, q_b, k_w, k_b, v_w, v_b, gamma, **run_kw):
    if "nc" not in _CACHED:
        _CACHED["nc"] = build_nc()
    nc = _CACHED["nc"]
    in_maps = _prep_in_maps(x, conv_w, conv_b, q_w, q_b, k_w, k_b, v_w, v_b,
                            gamma)
    res = run_bass_kernel_spmd(nc, in_maps, core_ids=list(range(8)), **run_kw)
    _CACHED["last_result"] = res
    out = np.empty((B, C, HW), np.float32)
    for core in range(8):
        b, hf = core // 2, core % 2
        oc = np.asarray(res.results[core]["outT"])  # [2048, 256]
        out[b, :, hf * NQ : (hf + 1) * NQ] = oc.T
    return out.reshape(B, C, H, W)
